# revision 1
# baseline (speedup 1.0000x reference)
"""GAT 2-layer kernel for Trainium2, 8 NeuronCores, dst-sharded.

Self-contained: hardcodes all shapes. Strategy:
  - Nodes partitioned by dst-ownership: core c owns nodes [c*12500,(c+1)*12500).
  - 3 SPMD launches:
      A: per-core table1 shard  = [fp16 h1 (128) | f32 el1 (4)] rows (512B)
      B: L1 edge phase (gather table1 full) -> selu -> table2 shard
         = [f32 h2 (64) | f32 el2 (1) | pad] rows (512B) + er2
      C: L2 edge phase (gather table2 full) -> final out rows
  - Edge aggregation: per 128-edge block, one-hot S matmul into PSUM
    accumulated per 128-node tile; softmax without max-subtraction
    (exp is safe for this data scale); division by the segment sum after
    aggregation.
  - dma_gather int16 indices => gather table split in 4 chunks of 25000
    rows; host packs nodes into tiles so each (tile, chunk) has <= 512
    edge slots (4 blocks of 128, padded with idx=0 / dstloc=-1).
"""

import sys

sys.path.insert(0, "/opt/trn_rl_repo")

import numpy as np

from concourse import bacc, mybir, tile
from concourse.bass_utils import run_bass_kernel_spmd
from concourse.masks import make_identity

P = 128
N_NODES = 100000
N_EDGES = 1600000
NCORES = 8
NPC = N_NODES // NCORES          # 12500 nodes per core
NEG = 0.2                        # leaky relu slope
CH = 4                           # gather chunks (int16 idx limit)
CHW = 25000                      # chunk width (nodes)
BPC = 4                          # blocks per (tile, chunk)
CAP = BPC * P                    # 512 edge slots per (tile, chunk)
SLOTS_T = CH * CAP               # 2048 slots per tile
NBLK_T = CH * BPC                # 16 blocks per tile
GRP = 4                          # tiles per gather instruction
NTA = (NPC + P - 1) // P         # 98 phase-A tiles
NPC_PAD = NTA * P                # 12544
SELU_L = 1.0507009873554805
SELU_A = 1.6732632423543772

SKIP = set()  # debug ablation: {"gather","S","w","mm","out"}

fp16 = mybir.dt.float16
fp32 = mybir.dt.float32
i16 = mybir.dt.int16


# ----------------------------------------------------------------- host prep
def _pack_nodes(deg):
    """Greedy-pack NPC nodes into tiles of <=128 nodes with per-chunk edge
    loads <= CAP. deg: [NPC, CH] int. Returns (node_tile, node_row, nt)."""
    total = deg.sum(1)
    order = np.argsort(-total, kind="stable")
    nt = NTA + 2
    while True:
        loads = np.zeros((nt, CH), np.int64)
        counts = np.zeros(nt, np.int64)
        node_tile = np.empty(NPC, np.int64)
        node_row = np.empty(NPC, np.int64)
        ok_all = True
        for n in order:
            d = deg[n]
            ok = (counts < P) & np.all(loads + d <= CAP, axis=1)
            if not ok.any():
                ok_all = False
                break
            # best-fit: among feasible, most loaded tile first (by total)
            cand = np.nonzero(ok)[0]
            t = cand[np.argmax(loads[cand].sum(1) + counts[cand])]
            node_tile[n] = t
            node_row[n] = counts[t]
            counts[t] += 1
            loads[t] += d
        if ok_all:
            return node_tile, node_row, nt
        nt += 2


def _host_prep(src, dst):
    """Edge/packing preprocessing for all cores. Returns per-core dict list
    and the common tile count NT."""
    owner = dst // NPC
    cores = []
    for c in range(NCORES):
        sel = np.nonzero(owner == c)[0]
        e_src = src[sel].astype(np.int64)
        e_dstloc = (dst[sel] - c * NPC).astype(np.int64)
        e_chunk = e_src // CHW
        deg = np.bincount(e_dstloc * CH + e_chunk, minlength=NPC * CH)
        deg = deg.reshape(NPC, CH)
        node_tile, node_row, nt = _pack_nodes(deg)
        cores.append(dict(e_src=e_src, e_dstloc=e_dstloc, e_chunk=e_chunk,
                          node_tile=node_tile, node_row=node_row, nt=nt))
    NT = max(cd["nt"] for cd in cores)
    NT = ((NT + GRP - 1) // GRP) * GRP

    for cd in cores:
        e_src, e_dstloc, e_chunk = cd["e_src"], cd["e_dstloc"], cd["e_chunk"]
        node_tile, node_row = cd["node_tile"], cd["node_row"]
        e_tile = node_tile[e_dstloc]
        e_row = node_row[e_dstloc]
        key = e_tile * CH + e_chunk
        order_e = np.argsort(key, kind="stable")
        key_s = key[order_e]
        gs = np.bincount(key_s, minlength=NT * CH)
        gstart = np.concatenate([[0], np.cumsum(gs)])[:-1]
        # position of each sorted edge within its (tile, chunk) group
        within = np.arange(len(key_s)) - gstart[key_s]
        assert within.max(initial=0) < CAP, "packing overflow"
        # global slot id = tile*SLOTS_T + chunk*CAP + within
        slot = key_s // CH * SLOTS_T + (key_s % CH) * CAP + within

        # slot-indexed arrays (pad: idx=0, dstloc=-1)
        nslot = NT * SLOTS_T
        s_idx = np.zeros(nslot, np.int16)
        s_dst = np.full(nslot, -1.0, np.float32)
        s_idx[slot] = (e_src[order_e] - e_chunk[order_e] * CHW).astype(np.int16)
        s_dst[slot] = e_row[order_e].astype(np.float32)
        s_node = np.full(nslot, -1, np.int64)
        s_node[slot] = e_dstloc[order_e]

        # idx_arr: gather (grp,c) = concat of GRP tiles' chunk-c 512-lists;
        # idx j -> (partition j%16 (replicated x8), col j//16)
        v = s_idx.reshape(NT // GRP, GRP, CH, CAP)
        v = np.ascontiguousarray(v.transpose(0, 2, 1, 3))  # [ngrp, c, g, cap]
        v = v.reshape(NT // GRP * CH, GRP * CAP // 16, 16)
        idx16 = np.ascontiguousarray(v.transpose(2, 0, 1)).reshape(16, -1)
        idx_arr = np.tile(idx16, (8, 1))          # [128, NT*CH*32]

        # dstrow [NT, SLOTS_T] fp16; dstcol [128, NT*16] f32
        dstrow = s_dst.reshape(NT, SLOTS_T).astype(np.float16)
        dc = s_dst.reshape(NT, NBLK_T, P)         # [t, b, p]
        dstcol = np.ascontiguousarray(dc.transpose(2, 0, 1)).reshape(P, -1)
        dstcol = dstcol.astype(np.float32)

        # packed-order -> global-node permutation (for er / table2 / out)
        # packed position q = tile*128 + row ; perm[q] = node id or -1
        perm = np.full(NT * P, -1, np.int64)
        perm[node_tile * P + node_row] = np.arange(NPC)
        cd.update(idx_arr=idx_arr, dstcol=dstcol, perm=perm, s_node=s_node)
    return cores, NT


# ------------------------------------------------------------------ launch A
def _build_launch_a():
    nc = bacc.Bacc("TRN2", target_bir_lowering=False, debug=False)
    xs = nc.dram_tensor("xs", [NPC_PAD, P], fp32, kind="ExternalInput")
    w1 = nc.dram_tensor("w1", [P, P], fp32, kind="ExternalInput")
    almat = nc.dram_tensor("almat", [P, 8], fp32, kind="ExternalInput")
    tab = nc.dram_tensor("tab", [NPC_PAD, 256], fp16, kind="ExternalOutput")
    er1 = nc.dram_tensor("er1", [P, NTA * 4], fp16, kind="ExternalOutput")

    with tile.TileContext(nc) as tc:
        with (
            tc.tile_pool(name="const", bufs=1) as cp,
            tc.tile_pool(name="sb", bufs=3) as sb,
            tc.tile_pool(name="ps", bufs=2, space="PSUM") as ps,
        ):
            ident = cp.tile([P, P], fp32)
            make_identity(nc, ident[:])
            w1_sb = cp.tile([P, P], fp32)
            nc.sync.dma_start(out=w1_sb[:], in_=w1[:])
            alm_sb = cp.tile([P, 8], fp32)
            nc.sync.dma_start(out=alm_sb[:], in_=almat[:])

            # W1T = transpose(W1); Wal8 = W1T.T-contract: [in,8]
            w1t_ps = ps.tile([P, P], fp32, tag="tp")
            nc.tensor.transpose(out=w1t_ps[:], in_=w1_sb[:], identity=ident[:])
            w1t_sb = cp.tile([P, P], fp32)
            nc.vector.tensor_copy(out=w1t_sb[:], in_=w1t_ps[:])
            rhsw = cp.tile([P, 136], fp32)
            nc.vector.tensor_copy(out=rhsw[:, 0:P], in_=w1_sb[:])
            wal_ps = ps.tile([P, 8], fp32, tag="wal")
            nc.tensor.matmul(out=wal_ps[:], lhsT=w1t_sb[:], rhs=alm_sb[:],
                             start=True, stop=True)
            nc.vector.tensor_copy(out=rhsw[:, P:136], in_=wal_ps[:])
            erall = cp.tile([P, NTA * 4], fp16)

            for t in range(NTA):
                xt = sb.tile([P, P], fp32, tag="x")
                nc.sync.dma_start(out=xt[:], in_=xs[t * P:(t + 1) * P, :])
                xT_ps = ps.tile([P, P], fp32, tag="tp")
                nc.tensor.transpose(out=xT_ps[:], in_=xt[:], identity=ident[:])
                xT = sb.tile([P, P], fp32, tag="xT")
                nc.vector.tensor_copy(out=xT[:], in_=xT_ps[:])
                hel = ps.tile([P, 136], fp32, tag="hel")
                nc.tensor.matmul(out=hel[:], lhsT=xT[:], rhs=rhsw[:],
                                 start=True, stop=True)
                row = sb.tile([P, 256], fp16, tag="row")
                nc.vector.tensor_copy(out=row[:, 0:P], in_=hel[:, 0:P])
                row32 = row[:].bitcast(fp32)
                nc.vector.tensor_copy(out=row32[:, 64:68], in_=hel[:, P:132])
                nc.vector.tensor_copy(out=erall[:, t * 4:(t + 1) * 4],
                                      in_=hel[:, 132:136])
                nc.sync.dma_start(out=tab[t * P:(t + 1) * P, :], in_=row[:])
            nc.sync.dma_start(out=er1[:], in_=erall[:])
    nc.compile()
    return nc


# --------------------------------------------------------- launch B/C common
def _edge_phase(nc, tc, cp, NT, gtab, gdt, fdim, idx_sb, dstcol_sb,
                iota_row, erx_sb, nheads, per_tile_out):
    """Shared L1/L2 edge machinery. fdim = feature cols (128 or 64).
    gtab: DRAM gather table [N_NODES, elem]; gdt its dtype.
    erx_sb: [128, NT*NBLK_T*nheads] fp16 per-edge-slot er (host-expanded).
    per_tile_out(t, num_ps, sm, ops_): consume [P, fdim+nheads] PSUM."""
    elem = 512 // mybir.dt.size(gdt)          # row elems (512B)
    with (
        tc.tile_pool(name="gb", bufs=3) as gb,
        tc.tile_pool(name="wb", bufs=2) as wb,
        tc.tile_pool(name="stp", bufs=3) as stp,
        tc.tile_pool(name="sm", bufs=3) as sm,
        tc.tile_pool(name="nps", bufs=2, space="PSUM") as nps,
        tc.tile_pool(name="ops", bufs=2, space="PSUM") as ops_,
    ):
        GW = GRP * CAP // 16                   # idx cols per grouped gather
        for t in range(NT):
            gi = t % GRP
            if gi == 0:
                grp = t // GRP
                gbuf = gb.tile([P, CH, GRP, BPC, elem], gdt, tag="g")
                for c in range(CH) if "gather" not in SKIP else []:
                    nc.gpsimd.dma_gather(
                        out_ap=gbuf[:, c].rearrange("p g b e -> p (g b) e"),
                        in_ap=gtab[c * CHW:(c + 1) * CHW, :],
                        idxs_ap=idx_sb[:, (grp * CH + c) * GW:
                                       (grp * CH + c + 1) * GW],
                        num_idxs=GRP * CAP,
                        num_idxs_reg=GRP * CAP,
                        elem_size=elem,
                        single_packet=False,
                        queue_num=c % 4,
                    )
            gt = gbuf[:, :, gi]                # [P, CH, BPC, elem]
            # e = el + er ; leaky ; exp
            g32 = gt.bitcast(fp32)             # [P, CH, BPC, 128]
            el_view = g32[:, :, :, 64:64 + nheads]
            ern = NBLK_T * nheads
            ea = sm.tile([P, CH, BPC, nheads], fp32, tag="ea")
            nc.vector.tensor_tensor(
                out=ea[:], in0=el_view,
                in1=erx_sb[:, t * ern:(t + 1) * ern].rearrange(
                    "p (c b h) -> p c b h", c=CH, h=nheads),
                op=mybir.AluOpType.add)
            eb = sm.tile([P, NBLK_T * nheads], fp32, tag="eb")
            nc.vector.tensor_scalar(out=eb[:],
                                    in0=ea[:].rearrange(
                                        "p c b h -> p (c b h)"),
                                    scalar1=NEG, scalar2=None,
                                    op0=mybir.AluOpType.mult)
            nc.vector.tensor_tensor(out=eb[:],
                                    in0=ea[:].rearrange(
                                        "p c b h -> p (c b h)"),
                                    in1=eb[:], op=mybir.AluOpType.max)
            ex = sm.tile([P, NBLK_T * nheads], fp32, tag="ex")
            nc.scalar.activation(out=ex[:], in_=eb[:],
                                 func=mybir.ActivationFunctionType.Exp)
            # w = h * ex  (+ ex cols appended), fp16
            w = wb.tile([P, NBLK_T, fdim + nheads], fp16, tag="w")
            if gdt == fp32:
                h_in = g32[:, :, :, 0:fdim]
            else:
                h_in = gt[:, :, :, 0:fdim]
            dph = fdim // nheads
            if "w" not in SKIP:
                nc.vector.tensor_tensor(
                    out=w[:, :, 0:fdim].rearrange(
                        "p (c b) (h d) -> p c b h d", c=CH, d=dph),
                    in0=h_in.rearrange("p c b (h d) -> p c b h d", d=dph),
                    in1=ex[:].rearrange("p (c b h) -> p c b h",
                                        c=CH, h=nheads)[
                        :, :, :, :, None].to_broadcast(
                            [P, CH, BPC, nheads, dph]),
                    op=mybir.AluOpType.mult,
                )
            nc.vector.tensor_copy(
                out=w[:, :, fdim:fdim + nheads],
                in_=ex[:].rearrange("p (b h) -> p b h", h=nheads))
            # one-hot matmuls, accumulate per tile
            num_ps = nps.tile([P, fdim + nheads], fp32, tag="num")
            sconst = None
            for b in range(NBLK_T):
                if "S" not in SKIP:
                    sblk = stp.tile([P, P], fp16, tag="sblk")
                    nc.vector.tensor_scalar(
                        out=sblk[:], in0=iota_row[:],
                        scalar1=dstcol_sb[:, t * NBLK_T + b:t * NBLK_T + b + 1],
                        scalar2=None, op0=mybir.AluOpType.is_equal)
                elif sconst is None:
                    sconst = stp.tile([P, P], fp16, tag="sblk")
                    nc.vector.tensor_copy(out=sconst[:], in_=iota_row[:])
                    sblk = sconst
                else:
                    sblk = sconst
                if "mm" not in SKIP:
                    nc.tensor.matmul(out=num_ps[:], lhsT=sblk[:],
                                     rhs=w[:, b, :],
                                     start=(b == 0), stop=(b == NBLK_T - 1))
            if "mm" in SKIP:
                nc.vector.tensor_copy(out=num_ps[:], in_=w[:, 0, :])
            if "out" not in SKIP:
                per_tile_out(t, num_ps, sm, ops_)


def _build_launch_b(NT):
    nc = bacc.Bacc("TRN2", target_bir_lowering=False, debug=False,
                   num_swdge_queues=4)
    tab1 = nc.dram_tensor("tab1", [CHW * CH, 256], fp16, kind="ExternalInput")
    erxd = nc.dram_tensor("erxd", [P, NT * NBLK_T * 4], fp16,
                          kind="ExternalInput")
    idxa = nc.dram_tensor("idxa", [P, NT * P], i16, kind="ExternalInput")
    dcd = nc.dram_tensor("dcd", [P, NT * NBLK_T], fp32, kind="ExternalInput")
    iar = nc.dram_tensor("iar", [P, P], fp16, kind="ExternalInput")
    w2 = nc.dram_tensor("w2", [P, 64], fp32, kind="ExternalInput")
    alar2 = nc.dram_tensor("alar2", [64, 2], fp32, kind="ExternalInput")
    tab2 = nc.dram_tensor("tab2", [NT * P, P], fp32, kind="ExternalOutput")
    er2 = nc.dram_tensor("er2", [P, NT], fp16, kind="ExternalOutput")

    with tile.TileContext(nc) as tc:
        with tc.tile_pool(name="const", bufs=1) as cp:
            ident = cp.tile([P, P], fp32)
            make_identity(nc, ident[:])
            idx_sb = cp.tile([P, NT * P], i16)
            nc.sync.dma_start(out=idx_sb[:], in_=idxa[:])
            dstcol_sb = cp.tile([P, NT * NBLK_T], fp32)
            nc.sync.dma_start(out=dstcol_sb[:], in_=dcd[:])
            iota_row = cp.tile([P, P], fp16)
            nc.sync.dma_start(out=iota_row[:], in_=iar[:])
            erx_sb = cp.tile([P, NT * NBLK_T * 4], fp16)
            nc.sync.dma_start(out=erx_sb[:], in_=erxd[:])
            er2all = cp.tile([P, NT], fp16)
            # W2rhs = [W2 | Wal2 | War2]
            w2_sb = cp.tile([P, 64], fp32)
            nc.sync.dma_start(out=w2_sb[:], in_=w2[:])
            al2_sb = cp.tile([64, 2], fp32)
            nc.sync.dma_start(out=al2_sb[:], in_=alar2[:])
            w2rhs = cp.tile([P, 66], fp32)
            nc.vector.tensor_copy(out=w2rhs[:, 0:64], in_=w2_sb[:])

            with tc.tile_pool(name="pre", bufs=1, space="PSUM") as pp:
                w2t_ps = pp.tile([64, P], fp32)
                nc.tensor.transpose(out=w2t_ps[:], in_=w2_sb[:],
                                    identity=ident[:])
                w2t_sb = cp.tile([64, P], fp32)
                nc.vector.tensor_copy(out=w2t_sb[:], in_=w2t_ps[:])
                wal2_ps = pp.tile([P, 2], fp32)
                nc.tensor.matmul(out=wal2_ps[:], lhsT=w2t_sb[:],
                                 rhs=al2_sb[:], start=True, stop=True)
                nc.vector.tensor_copy(out=w2rhs[:, 64:66], in_=wal2_ps[:])

            def out_b(t, num_ps, sm, ops_):
                # h1out = num/s -> selu -> table2 row + er2
                rec = sm.tile([P, 4], fp32, tag="rec")
                smax = sm.tile([P, 4], fp32, tag="smax")
                nc.vector.tensor_scalar(out=smax[:], in0=num_ps[:, 128:132],
                                        scalar1=1e-30, scalar2=None,
                                        op0=mybir.AluOpType.max)
                nc.vector.reciprocal(out=rec[:], in_=smax[:])
                h1o = sm.tile([P, P], fp32, tag="h1o")
                nc.vector.tensor_tensor(
                    out=h1o[:].rearrange("p (h d) -> p h d", d=32),
                    in0=num_ps[:, 0:128].rearrange("p (h d) -> p h d", d=32),
                    in1=rec[:][:, :, None].to_broadcast([P, 4, 32]),
                    op=mybir.AluOpType.mult)
                # selu
                m0 = sm.tile([P, P], fp32, tag="m0")
                nc.vector.tensor_scalar(out=m0[:], in0=h1o[:], scalar1=0.0,
                                        scalar2=None, op0=mybir.AluOpType.min)
                ew = sm.tile([P, P], fp32, tag="ew")
                nc.scalar.activation(out=ew[:], in_=m0[:],
                                     func=mybir.ActivationFunctionType.Exp)
                pos = sm.tile([P, P], fp32, tag="pos")
                nc.vector.tensor_scalar(out=pos[:], in0=h1o[:], scalar1=0.0,
                                        scalar2=SELU_L,
                                        op0=mybir.AluOpType.max,
                                        op1=mybir.AluOpType.mult)
                h1p = sm.tile([P, P], fp32, tag="h1p")
                nc.vector.scalar_tensor_tensor(
                    out=h1p[:], in0=ew[:], scalar=SELU_L * SELU_A,
                    in1=pos[:], op0=mybir.AluOpType.mult,
                    op1=mybir.AluOpType.add)
                nc.vector.tensor_scalar(out=h1p[:], in0=h1p[:],
                                        scalar1=SELU_L * SELU_A, scalar2=None,
                                        op0=mybir.AluOpType.subtract)
                # table2: h2el = (h1p)^T-matmul W2rhs
                h1t_ps = ops_.tile([P, P], fp32, tag="h1t")
                nc.tensor.transpose(out=h1t_ps[:], in_=h1p[:],
                                    identity=ident[:])
                h1t = sm.tile([P, P], fp32, tag="h1t_sb")
                nc.vector.tensor_copy(out=h1t[:], in_=h1t_ps[:])
                h2el = ops_.tile([P, 66], fp32, tag="h2el")
                nc.tensor.matmul(out=h2el[:], lhsT=h1t[:], rhs=w2rhs[:],
                                 start=True, stop=True)
                trow = sm.tile([P, P], fp32, tag="trow")
                nc.gpsimd.memset(trow[:, 65:128], 0)
                nc.vector.tensor_copy(out=trow[:, 0:65], in_=h2el[:, 0:65])
                nc.vector.tensor_copy(out=er2all[:, t:t + 1],
                                      in_=h2el[:, 65:66])
                nc.sync.dma_start(out=tab2[t * P:(t + 1) * P, :], in_=trow[:])

            _edge_phase(nc, tc, cp, NT, tab1, fp16, 128, idx_sb,
                        dstcol_sb, iota_row, erx_sb, 4, out_b)
            nc.sync.dma_start(out=er2[:], in_=er2all[:])
    nc.compile()
    return nc


def _build_launch_c(NT):
    nc = bacc.Bacc("TRN2", target_bir_lowering=False, debug=False,
                   num_swdge_queues=4)
    tab2 = nc.dram_tensor("tab2", [CHW * CH, P], fp32, kind="ExternalInput")
    erxd = nc.dram_tensor("erxd", [P, NT * NBLK_T], fp16,
                          kind="ExternalInput")
    idxa = nc.dram_tensor("idxa", [P, NT * P], i16, kind="ExternalInput")
    dcd = nc.dram_tensor("dcd", [P, NT * NBLK_T], fp32, kind="ExternalInput")
    iar = nc.dram_tensor("iar", [P, P], fp16, kind="ExternalInput")
    outp = nc.dram_tensor("outp", [NT * P, 64], fp32, kind="ExternalOutput")

    with tile.TileContext(nc) as tc:
        with tc.tile_pool(name="const", bufs=1) as cp:
            idx_sb = cp.tile([P, NT * P], i16)
            nc.sync.dma_start(out=idx_sb[:], in_=idxa[:])
            dstcol_sb = cp.tile([P, NT * NBLK_T], fp32)
            nc.sync.dma_start(out=dstcol_sb[:], in_=dcd[:])
            iota_row = cp.tile([P, P], fp16)
            nc.sync.dma_start(out=iota_row[:], in_=iar[:])
            erx_sb = cp.tile([P, NT * NBLK_T], fp16)
            nc.sync.dma_start(out=erx_sb[:], in_=erxd[:])

            def out_c(t, num_ps, sm, ops_):
                rec = sm.tile([P, 1], fp32, tag="rec")
                smax = sm.tile([P, 1], fp32, tag="smax")
                nc.vector.tensor_scalar(out=smax[:], in0=num_ps[:, 64:65],
                                        scalar1=1e-30, scalar2=None,
                                        op0=mybir.AluOpType.max)
                nc.vector.reciprocal(out=rec[:], in_=smax[:])
                oo = sm.tile([P, 64], fp32, tag="oo")
                nc.vector.tensor_tensor(
                    out=oo[:], in0=num_ps[:, 0:64],
                    in1=rec[:].to_broadcast([P, 64]),
                    op=mybir.AluOpType.mult)
                nc.sync.dma_start(out=outp[t * P:(t + 1) * P, :], in_=oo[:])

            _edge_phase(nc, tc, cp, NT, tab2, fp32, 64, idx_sb,
                        dstcol_sb, iota_row, erx_sb, 1, out_c)
    nc.compile()
    return nc


# ------------------------------------------------------------------- driver
_info = {}


def _run(nc, in_maps, tries=3):
    import time
    last = None
    for i in range(tries):
        try:
            return run_bass_kernel_spmd(nc, in_maps, list(range(NCORES)))
        except Exception as e:  # flaky NRT_EXEC_UNIT_UNRECOVERABLE
            last = e
            print(f"run attempt {i} failed: {e}", flush=True)
            time.sleep(5)
    raise last


def kernel(x, src, dst, W1, al1, ar1, W2, al2, ar2):
    import time
    x = np.asarray(x)
    src = np.asarray(src)
    dst = np.asarray(dst)
    W1 = np.asarray(W1, np.float32)
    al1 = np.asarray(al1, np.float32)
    ar1 = np.asarray(ar1, np.float32)
    W2 = np.asarray(W2, np.float32)
    al2 = np.asarray(al2, np.float32)
    ar2 = np.asarray(ar2, np.float32)

    t0 = time.time()
    cores, NT = _host_prep(src, dst)
    _info["prep_s"] = time.time() - t0
    _info["NT"] = NT

    # --- launch A
    almat = np.zeros((P, 8), np.float32)
    for h in range(4):
        almat[32 * h:32 * (h + 1), h] = al1[h]
        almat[32 * h:32 * (h + 1), 4 + h] = ar1[h]
    nc_a = _build_launch_a()
    in_a = []
    for c in range(NCORES):
        xs = np.zeros((NPC_PAD, P), np.float32)
        xs[:NPC] = x[c * NPC:(c + 1) * NPC]
        in_a.append({"xs": xs, "w1": W1, "almat": almat})
    ra = _run(nc_a, in_a)

    tab1 = np.concatenate([ra.results[c]["tab"][:NPC] for c in range(NCORES)])
    er1 = []
    for c in range(NCORES):
        e = ra.results[c]["er1"].reshape(P, NTA, 4)       # [p, t, h]
        er1.append(np.ascontiguousarray(
            e.transpose(1, 0, 2)).reshape(NPC_PAD, 4)[:NPC])

    # --- launch B
    iar = np.broadcast_to(np.arange(P, dtype=np.float16), (P, P)).copy()
    alar2 = np.stack([al2[0], ar2[0]], axis=1).astype(np.float32)
    nc_b = _build_launch_b(NT)

    def _erx(cd, er_glob, nh):
        """er per edge-slot, layout [128, NT*NBLK_T*nh] fp16."""
        sn = cd["s_node"].reshape(NT, NBLK_T, P)
        er = np.zeros((NT, NBLK_T, P, nh), np.float16)
        valid = sn >= 0
        er[valid] = er_glob[sn[valid]]
        return np.ascontiguousarray(
            er.transpose(2, 0, 1, 3)).reshape(P, NT * NBLK_T * nh)

    in_b = []
    for c in range(NCORES):
        cd = cores[c]
        in_b.append({"tab1": tab1, "erxd": _erx(cd, er1[c], 4),
                     "idxa": cd["idx_arr"], "dcd": cd["dstcol"],
                     "iar": iar, "w2": W2, "alar2": alar2})
    rb = _run(nc_b, in_b)

    # assemble table2 (global node order) + er2 per-slot inputs
    tab2 = np.zeros((N_NODES, P), np.float32)
    er2g = []
    for c in range(NCORES):
        cd = cores[c]
        perm = cd["perm"]
        valid = perm >= 0
        t2 = rb.results[c]["tab2"]              # packed order
        tab2[c * NPC + perm[valid]] = t2[valid]
        e2p = np.ascontiguousarray(
            rb.results[c]["er2"].transpose(1, 0)).reshape(NT * P, 1)
        e2 = np.zeros((NPC, 1), np.float16)
        e2[perm[valid], 0] = e2p[valid, 0]
        er2g.append(e2)

    # --- launch C
    nc_c = _build_launch_c(NT)
    in_c = []
    for c in range(NCORES):
        cd = cores[c]
        in_c.append({"tab2": tab2, "erxd": _erx(cd, er2g[c], 1),
                     "idxa": cd["idx_arr"], "dcd": cd["dstcol"], "iar": iar})
    rc_ = _run(nc_c, in_c)

    out = np.zeros((N_NODES, 64), np.float32)
    for c in range(NCORES):
        cd = cores[c]
        perm = cd["perm"]
        valid = perm >= 0
        op = rc_.results[c]["outp"]
        out[c * NPC + perm[valid]] = op[valid]

    _info["ncs"] = (nc_a, nc_b, nc_c)
    return out

